# revision 14
# baseline (speedup 1.0000x reference)
"""Trainium2 Bass kernel for nn_ChunkedCrossAttention_85907935855128.

Self-contained: hardcodes shapes/sharding. Accepts FULL inputs, returns FULL output.
Shards the fused (b*k_chunks) chunk axis across 8 NeuronCores; weights replicated.

Per-core dataflow (all matmul layouts chosen so no on-device transposition of the
big activations is needed; host passes x/context pre-transposed, dim-major):
  qT/kT inner-major via fp32r matmuls (lhsT=W tile, rhs=xT/ctxT), v token-major
  (lhsT=ctxT tile, rhs=Wv). Rope on k = cos*k + sin*(signed-perm matmul on PE).
  Rope on q is identity except each chunk's token 0 (causal shift zeroes the rest
  of the shifted q_pos_emb). Attention in bf16: simT[j,(h,i)] psum -> ACT exp ->
  o[i,65] psum (col 64 = softmax sum via ones column in v_aug) -> reciprocal *
  per-head -> PE-transpose -> fp32r out-projection + bias.
"""
import os
# bass2jax executes via the axon PJRT platform; a CPU pin would hide the cores.
if os.environ.get("JAX_PLATFORMS", "") in ("cpu",):
    del os.environ["JAX_PLATFORMS"]

import numpy as np

import concourse.bacc as bacc
import concourse.bass as bass
import concourse.mybir as mybir
import concourse.tile as tile
from concourse.bass_utils import run_bass_kernel_spmd
from concourse.masks import make_identity

F32 = mybir.dt.float32
F32R = mybir.dt.float32r
BF16 = mybir.dt.bfloat16

CS, CP, H, DH = 64, 63, 8, 64
SCALE = DH ** -0.5
N_CORES = 8
B, N, DIM = 4, 4096, 1024
K_CHUNKS, R, RLEN = 64, 2, 128
TK = R * RLEN                 # 256 ctx tokens / chunk
BK = B * K_CHUNKS             # 256 chunks
CPC = BK // N_CORES           # 32 chunks / core
TQ = CPC * CS                 # 2048 q tokens / core
TCTX = CPC * TK               # 8192 ctx tokens / core
INNER = H * DH                # 512
QG = 4                        # chunks per q-projection group (N=256)
NQG = CPC // QG               # 8 q groups / core


def _build_bass(cpc=CPC, num_devices=N_CORES, do_rope=True, do_attn=True, do_out=True, attn_stop=3):
    tq = cpc * CS
    tctx = cpc * TK
    nqg = cpc // QG
    nc = bacc.Bacc("TRN2", target_bir_lowering=False, debug=False,
                   num_devices=num_devices)

    xT = nc.dram_tensor("xT", (DIM, tq), F32, kind="ExternalInput")
    ctxT = nc.dram_tensor("ctxT", (DIM, tctx), F32, kind="ExternalInput")
    Wq = nc.dram_tensor("Wq", (DIM, INNER), F32, kind="ExternalInput")   # pre-scaled
    Wk = nc.dram_tensor("Wk", (DIM, INNER), F32, kind="ExternalInput")
    Wv = nc.dram_tensor("Wv", (DIM, INNER), F32, kind="ExternalInput")
    Wo = nc.dram_tensor("Wo", (INNER, DIM), F32, kind="ExternalInput")
    bo = nc.dram_tensor("bo", (DIM,), F32, kind="ExternalInput")
    cos_kT = nc.dram_tensor("cos_kT", (64, 128), F32, kind="ExternalInput")
    sin_kT = nc.dram_tensor("sin_kT", (64, 128), F32, kind="ExternalInput")
    Pm = nc.dram_tensor("Pm", (64, 64), F32, kind="ExternalInput")
    nullkT = nc.dram_tensor("nullkT", (64, 8), F32, kind="ExternalInput")
    nullv_aug = nc.dram_tensor("nullv_aug", (1, 8 * 65), F32, kind="ExternalInput")
    cos_q0 = nc.dram_tensor("cos_q0", (64, 1), F32, kind="ExternalInput")
    sin_q0s = nc.dram_tensor("sin_q0s", (64, 1), F32, kind="ExternalInput")
    out = nc.dram_tensor("out", (tq, DIM), F32, kind="ExternalOutput")

    with tile.TileContext(nc) as tc:
        with tc.tile_pool(name="consts", bufs=1) as cp_, \
             tc.tile_pool(name="wk", bufs=2) as wk, \
             tc.tile_pool(name="psb", bufs=3, space="PSUM") as psb, \
             tc.tile_pool(name="pst", bufs=1, space="PSUM") as pst:

            # ---- constants ----
            wq_sb = cp_.tile([128, 8, INNER], F32R)
            nc.sync.dma_start(out=wq_sb, in_=Wq[:, :].rearrange(
                "(dt p) i -> p dt i", p=128).bitcast(F32R))
            wk_sb = cp_.tile([128, 8, INNER], F32R)
            nc.sync.dma_start(out=wk_sb, in_=Wk[:, :].rearrange(
                "(dt p) i -> p dt i", p=128).bitcast(F32R))
            wv_sb = cp_.tile([128, 8, INNER], F32R)
            nc.sync.dma_start(out=wv_sb, in_=Wv[:, :].rearrange(
                "(dt p) i -> p dt i", p=128).bitcast(F32R))
            wo_sb = cp_.tile([128, 4, DIM], F32R)
            nc.sync.dma_start(out=wo_sb, in_=Wo[:, :].rearrange(
                "(et p) c -> p et c", p=128).bitcast(F32R))

            bo_sb = cp_.tile([64, DIM], F32)
            nc.sync.dma_start(out=bo_sb, in_=bass.AP(
                tensor=bo, offset=0, ap=[[0, 64], [1, DIM]]))

            cosk_sb = cp_.tile([64, 128], F32)
            nc.sync.dma_start(out=cosk_sb, in_=cos_kT[:, :])
            sink_sb = cp_.tile([64, 128], F32)
            nc.sync.dma_start(out=sink_sb, in_=sin_kT[:, :])
            cosq_sb = cp_.tile([64, 1], F32)
            nc.sync.dma_start(out=cosq_sb, in_=cos_q0[:, :])
            sinq_sb = cp_.tile([64, 1], F32)
            nc.sync.dma_start(out=sinq_sb, in_=sin_q0s[:, :])

            pm_f32 = cp_.tile([64, 64], F32)
            nc.sync.dma_start(out=pm_f32, in_=Pm[:, :])
            pm_bf = cp_.tile([64, 64], BF16)
            nc.vector.tensor_copy(pm_bf, pm_f32)

            nullk_f32 = cp_.tile([64, 8], F32)
            nc.sync.dma_start(out=nullk_f32, in_=nullkT[:, :])
            nullk_bf = cp_.tile([64, 8], BF16)
            nc.vector.tensor_copy(nullk_bf, nullk_f32)

            nullv_f32 = cp_.tile([1, 8, 65], F32)
            nc.sync.dma_start(out=nullv_f32, in_=nullv_aug[:, :].rearrange(
                "o (h w) -> o h w", h=8))
            nullv_bf = cp_.tile([1, 8, 65], BF16)
            nc.vector.tensor_copy(nullv_bf, nullv_f32)

            ident = cp_.tile([128, 128], F32)
            make_identity(nc, ident)

            for g in range(nqg):          # 8 groups of 4 chunks
                # ---- q projection for this group: qT [512, 256] ----
                xT_sb = wk.tile([128, 8, QG * CS], F32R, tag="xT", bufs=1)
                nc.sync.dma_start(out=xT_sb, in_=xT[:, :].rearrange(
                    "(dt p) t -> p dt t", p=128)[:, :, g * QG * CS:(g + 1) * QG * CS]
                    .bitcast(F32R))
                qps = psb.tile([128, 4, QG * CS], F32, tag="ps", name=f"qps{g}")
                for it in range(4):
                    for dt in range(8):
                        nc.tensor.matmul(
                            qps[:, it, :],
                            wq_sb[:, dt, it * 128:(it + 1) * 128],
                            xT_sb[:, dt, :],
                            start=(dt == 0), stop=(dt == 7))
                qT_sb = wk.tile([64, 8, QG * CS], BF16, tag="qT", bufs=2)
                for it in range(4):
                    nc.vector.tensor_copy(qT_sb[:, 2 * it, :], qps[0:64, it, :])
                    nc.vector.tensor_copy(qT_sb[:, 2 * it + 1, :], qps[64:128, it, :])
                # rope-q: fix token 0 of each chunk (cols ::CS)
                qcols = qT_sb[:, :, :].rearrange(
                    "p h (c w) -> p h c w", w=CS)[:, :, :, 0]   # [64, 8, QG]
                t1q = wk.tile([64, 8, QG], BF16, tag="t1q", bufs=2)
                nc.vector.tensor_mul(
                    t1q, qcols,
                    cosq_sb.unsqueeze(2).broadcast_to((64, 8, QG)))
                t2q = wk.tile([64, 8, QG], BF16, tag="t2q", bufs=2)
                for (dst, src) in ((0, 32), (32, 0)):
                    nc.vector.tensor_mul(
                        t2q[dst:dst + 32, :, :],
                        qT_sb[:, :, :].rearrange(
                            "p h (c w) -> p h c w", w=CS)[src:src + 32, :, :, 0],
                        sinq_sb[src:src + 32, :].unsqueeze(2)
                        .broadcast_to((32, 8, QG)))
                nc.vector.tensor_add(qcols, t1q, t2q)

                # ---- null sims for group: expn_g [1, 8, 256] bf16 ----
                expn_g = wk.tile([1, 8, QG * CS], BF16, tag="expn", bufs=2)
                for h in range(H):
                    nps = pst.tile([1, QG * CS], F32, tag="pst", name=f"nps{g}_{h}")
                    nc.tensor.matmul(
                        nps[:, :],
                        nullk_bf[:, h:h + 1],
                        qT_sb[:, h, :],
                        start=True, stop=True)
                    nc.scalar.activation(expn_g[:, h, :], nps[:, :],
                                         mybir.ActivationFunctionType.Exp)

                for cc in range(QG):
                    c = g * QG + cc       # chunk index within core
                    # ---- load ctxT slice [1024, 256] ----
                    ctx_sb = wk.tile([128, 8, TK], F32R, tag="ctx", bufs=2)
                    nc.sync.dma_start(out=ctx_sb, in_=ctxT[:, :].rearrange(
                        "(dt p) t -> p dt t", p=128)[:, :, c * TK:(c + 1) * TK]
                        .bitcast(F32R))

                    # ---- k projection -> kraw bf16 ----
                    kps = psb.tile([128, 4, TK], F32, tag="ps", name=f"kps{c}")
                    for it in range(4):
                        for dt in range(8):
                            nc.tensor.matmul(
                                kps[:, it, :],
                                wk_sb[:, dt, it * 128:(it + 1) * 128],
                                ctx_sb[:, dt, :],
                                start=(dt == 0), stop=(dt == 7))
                    kraw = wk.tile([64, 8, TK], BF16, tag="kraw", bufs=2)
                    for it in range(4):
                        nc.scalar.copy(kraw[:, 2 * it, :], kps[0:64, it, :])
                        nc.scalar.copy(kraw[:, 2 * it + 1, :], kps[64:128, it, :])

                    # ---- rope-k: perm matmul + combine ----
                    if not do_rope:
                        kT_bf = kraw
                    else:
                      kpps_a = psb.tile([64, 4, TK], F32, tag="ps", name=f"kppsa{c}")
                      kpps_b = psb.tile([64, 4, TK], F32, tag="ps", name=f"kppsb{c}")
                      for q4 in range(4):
                        dst_t = (kpps_a, kpps_b)[q4 // 2]
                        nc.tensor.matmul(
                            dst_t[:, :, :].rearrange("p h t -> p (h t)")
                            [:, (q4 % 2) * 512:(q4 % 2 + 1) * 512],
                            pm_bf,
                            kraw[:, :, :].rearrange("p h t -> p (h t)")
                            [:, q4 * 512:(q4 + 1) * 512],
                            start=True, stop=True)
                      t1k = wk.tile([64, 8, TK], BF16, tag="t1k", bufs=1)
                      nc.vector.tensor_mul(
                        t1k[:, :, :].rearrange("p h (rep c) -> p h rep c", rep=2),
                        kraw[:, :, :].rearrange("p h (rep c) -> p h rep c", rep=2),
                        cosk_sb.unsqueeze(1).unsqueeze(2)
                        .broadcast_to((64, 8, 2, 128)))
                      t2k = wk.tile([64, 8, TK], BF16, tag="t2k", bufs=1)
                      for half, kp_t in ((0, kpps_a), (1, kpps_b)):
                        nc.vector.tensor_mul(
                            t2k[:, half * 4:(half + 1) * 4, :].rearrange(
                                "p h (rep c) -> p h rep c", rep=2),
                            kp_t[:, :, :].rearrange(
                                "p h (rep c) -> p h rep c", rep=2),
                            sink_sb.unsqueeze(1).unsqueeze(2)
                            .broadcast_to((64, 4, 2, 128)))
                      kT_bf = wk.tile([64, 8, TK], BF16, tag="kT", bufs=2)
                      nc.vector.tensor_add(kT_bf, t1k, t2k)

                    # ---- v projection -> v_aug bf16 [128, 2, 8, 65] ----
                    vps = psb.tile([128, 2, INNER], F32, tag="ps", name=f"vps{c}")
                    for tg in range(2):
                        for dt in range(8):
                            nc.tensor.matmul(
                                vps[:, tg, :],
                                ctx_sb[:, dt, tg * 128:(tg + 1) * 128],
                                wv_sb[:, dt, :],
                                start=(dt == 0), stop=(dt == 7))
                    v_aug = wk.tile([128, 2, 8, 65], BF16, tag="v_aug", bufs=2)
                    nc.scalar.copy(
                        v_aug[:, :, :, 0:64],
                        vps[:, :, :].rearrange("p tg (h w) -> p tg h w", h=8))
                    nc.gpsimd.memset(v_aug[:, :, :, 64:65], 1.0)

                    if not do_attn:
                        continue
                    # ---- sim matmuls: simT [128j, 2jg, (h,i)] ----
                    sps = psb.tile([128, 2, 512], F32, tag="ps", name=f"sps{c}")
                    for h in range(H):
                        for jg in range(2):
                            nc.tensor.matmul(
                                sps[:, jg, h * 64:(h + 1) * 64],
                                kT_bf[:, h, jg * 128:(jg + 1) * 128],
                                qT_sb[:, h, cc * CS:(cc + 1) * CS],
                                start=True, stop=True)
                    if attn_stop == 0:
                        dbg = wk.tile([64, DIM], F32, tag="out_sb", bufs=2)
                        nc.vector.tensor_copy(dbg[:, 0:512], sps[0:64, 0, :])
                        nc.vector.memset(dbg[:, 512:], 0.0)
                        nc.sync.dma_start(out=out[c * CS:(c + 1) * CS, :], in_=dbg)
                        continue
                    expT = wk.tile([128, 2, 512], BF16, tag="expT", bufs=2)
                    nc.scalar.activation(expT, sps,
                                         mybir.ActivationFunctionType.Exp)
                    if attn_stop == 1:
                        dbg = wk.tile([64, DIM], F32, tag="out_sb", bufs=2)
                        nc.vector.tensor_copy(dbg[:, 0:512], expT[0:64, 0, :])
                        nc.vector.memset(dbg[:, 512:], 0.0)
                        nc.sync.dma_start(out=out[c * CS:(c + 1) * CS, :], in_=dbg)
                        continue

                    # ---- o matmuls [64i, 65] per head (col 64 = softmax sum) ----
                    ops_ = psb.tile([64, 8, 128], F32, tag="ps", name=f"ops{c}")
                    for h in range(H):
                        dst = ops_[:, h, 0:65]
                        for jg in range(2):
                            nc.tensor.matmul(
                                dst,
                                expT[:, jg, h * 64:(h + 1) * 64],
                                v_aug[:, jg, h, :],
                                start=(jg == 0), stop=False)
                        nc.tensor.matmul(
                            dst,
                            expn_g[0:1, h, c * CS - g * QG * CS:
                                   c * CS - g * QG * CS + CS],
                            nullv_bf[0:1, h, :],
                            start=False, stop=True)

                    if attn_stop == 2:
                        dbg = wk.tile([64, DIM], F32, tag="out_sb", bufs=2)
                        nc.vector.tensor_copy(dbg[:, 0:128], ops_[:, 0, :])
                        nc.vector.memset(dbg[:, 128:], 0.0)
                        nc.sync.dma_start(out=out[c * CS:(c + 1) * CS, :], in_=dbg)
                        continue
                    # ---- normalize: o_sb [64, 8, 64] fp32 ----
                    rcol = wk.tile([64, 8], F32, tag="rcol", bufs=2)
                    nc.vector.reciprocal(rcol, ops_[:, :, 64])
                    o_sb = wk.tile([64, 8, 64], F32, tag="o_sb", bufs=2)
                    for h in range(H):
                        nc.vector.tensor_scalar_mul(
                            o_sb[:, h, :],
                            ops_[:, h, 0:64],
                            rcol[:, h:h + 1])

                    if not do_out:
                        continue
                    # ---- transpose o -> oT fp32r [128e, 4et, 64t] ----
                    otr = pst.tile([128, 4, 64], F32, tag="pst", name=f"otr{c}")
                    for et in range(4):
                        nc.tensor.transpose(
                            otr[:, et, :],
                            o_sb[:, 2 * et:2 * et + 2, :],
                            ident[0:64, 0:64])
                    oT_sb = wk.tile([128, 4, 64], F32R, tag="oT", bufs=2)
                    nc.vector.tensor_copy(oT_sb, otr)

                    # ---- out projection + bias ----
                    outps = psb.tile([64, DIM], F32, tag="ps", name=f"outps{c}")
                    for co in range(2):
                        for et in range(4):
                            nc.tensor.matmul(
                                outps[:, co * 512:(co + 1) * 512],
                                oT_sb[:, et, :],
                                wo_sb[:, et, co * 512:(co + 1) * 512],
                                start=(et == 0), stop=(et == 3))
                    out_sb = wk.tile([64, DIM], F32, tag="out_sb", bufs=2)
                    nc.vector.tensor_add(out_sb, outps, bo_sb)
                    nc.sync.dma_start(out=out[c * CS:(c + 1) * CS, :], in_=out_sb)

    nc.compile()
    return nc


_CACHED_NC = None


def _get_nc():
    global _CACHED_NC
    if _CACHED_NC is None:
        _CACHED_NC = _build_bass()
    return _CACHED_NC


def kernel(x, context, q_pos_emb, k_pos_emb, Wq, Wk, Wv, Wo, bo, null_k, null_v):
    x = np.asarray(x, dtype=np.float32)
    context = np.asarray(context, dtype=np.float32)
    q_pos_emb = np.asarray(q_pos_emb, dtype=np.float32)
    k_pos_emb = np.asarray(k_pos_emb, dtype=np.float32)
    Wq = np.asarray(Wq, dtype=np.float32)
    Wk = np.asarray(Wk, dtype=np.float32)
    Wv = np.asarray(Wv, dtype=np.float32)
    Wo = np.asarray(Wo, dtype=np.float32)
    bo = np.asarray(bo, dtype=np.float32)
    null_k = np.asarray(null_k, dtype=np.float32)
    null_v = np.asarray(null_v, dtype=np.float32)

    # ---- host marshalling (layout only + tiny rope tables) ----
    xs = np.zeros_like(x)
    xs[:, : N - CP] = x[:, CP:]
    xc = xs.reshape(BK, CS, DIM)
    ctx = context.reshape(BK, TK, DIM)

    Wq_s = np.ascontiguousarray(Wq * SCALE)

    qpe63 = q_pos_emb[0, 0, CP]
    cos_q0 = np.cos(qpe63)[:, None].astype(np.float32)          # [64, 1]
    sgn = np.where(np.arange(64) < 32, -1.0, 1.0)
    sin_q0s = (np.sin(qpe63) * sgn)[:, None].astype(np.float32)
    # permuted so the partition-shifted mul reads table at the src base
    # partition (BIR requires equal base partitions for two SBUF inputs)
    sp = np.empty_like(sin_q0s)
    sp[0:32] = sin_q0s[32:64]; sp[32:64] = sin_q0s[0:32]
    sin_q0s = sp

    kpe = k_pos_emb[0, 0]
    cos_kT = np.ascontiguousarray(np.cos(kpe.T).astype(np.float32))   # [64, 128]
    sin_kT = np.ascontiguousarray(np.sin(kpe.T).astype(np.float32))

    Pm = np.zeros((64, 64), np.float32)
    for rout in range(64):
        if rout < 32:
            Pm[rout + 32, rout] = -1.0
        else:
            Pm[rout - 32, rout] = 1.0

    nullkT = np.ascontiguousarray(null_k.reshape(8, 64).T.astype(np.float32))  # [64, 8]
    nullv_aug = np.zeros((1, 8, 65), np.float32)
    nullv_aug[0, :, :64] = null_v.reshape(8, 64)
    nullv_aug[0, :, 64] = 1.0
    nullv_aug = nullv_aug.reshape(1, 8 * 65)

    shared = {
        "Wq": Wq_s, "Wk": Wk, "Wv": Wv, "Wo": Wo, "bo": bo,
        "cos_kT": cos_kT, "sin_kT": sin_kT, "Pm": Pm,
        "nullkT": nullkT, "nullv_aug": nullv_aug,
        "cos_q0": cos_q0, "sin_q0s": sin_q0s,
    }
    in_maps = []
    for c in range(N_CORES):
        sl = slice(c * CPC, (c + 1) * CPC)
        xT_c = np.ascontiguousarray(xc[sl].reshape(TQ, DIM).T)
        ctxT_c = np.ascontiguousarray(ctx[sl].reshape(TCTX, DIM).T)
        in_maps.append({"xT": xT_c, "ctxT": ctxT_c, **shared})

    nc = _get_nc()
    res = run_bass_kernel_spmd(nc, in_maps, core_ids=list(range(N_CORES)))

    out_full = np.concatenate([res.results[c]["out"] for c in range(N_CORES)],
                              axis=0)                      # [BK*CS, DIM]
    o = out_full.reshape(B, K_CHUNKS * CS, DIM)
    final = np.concatenate(
        [np.zeros((B, CP, DIM), np.float32), o[:, : K_CHUNKS * CS - CP]], axis=1)
    return final


# revision 15
# speedup vs baseline: 1.1824x; 1.1824x over previous
"""Trainium2 Bass kernel for nn_ChunkedCrossAttention_85907935855128.

Self-contained: hardcodes shapes/sharding. Accepts FULL inputs, returns FULL output.
Shards the fused (b*k_chunks) chunk axis across 8 NeuronCores; weights replicated.

Per-core dataflow (all matmul layouts chosen so no on-device transposition of the
big activations is needed; host passes x/context pre-transposed, dim-major):
  qT/kT inner-major via fp32r matmuls (lhsT=W tile, rhs=xT/ctxT), v token-major
  (lhsT=ctxT tile, rhs=Wv). Rope on k = cos*k + sin*(signed-perm matmul on PE).
  Rope on q is identity except each chunk's token 0 (causal shift zeroes the rest
  of the shifted q_pos_emb). Attention in bf16: simT[j,(h,i)] psum -> ACT exp ->
  o[i,65] psum (col 64 = softmax sum via ones column in v_aug) -> reciprocal *
  per-head -> PE-transpose -> fp32r out-projection + bias.
"""
import os
# bass2jax executes via the axon PJRT platform; a CPU pin would hide the cores.
if os.environ.get("JAX_PLATFORMS", "") in ("cpu",):
    del os.environ["JAX_PLATFORMS"]

import numpy as np

import concourse.bacc as bacc
import concourse.bass as bass
import concourse.mybir as mybir
import concourse.tile as tile
from concourse.bass_utils import run_bass_kernel_spmd
from concourse.masks import make_identity

F32 = mybir.dt.float32
F32R = mybir.dt.float32r
BF16 = mybir.dt.bfloat16

CS, CP, H, DH = 64, 63, 8, 64
SCALE = DH ** -0.5
N_CORES = 8
B, N, DIM = 4, 4096, 1024
K_CHUNKS, R, RLEN = 64, 2, 128
TK = R * RLEN                 # 256 ctx tokens / chunk
BK = B * K_CHUNKS             # 256 chunks
CPC = BK // N_CORES           # 32 chunks / core
TQ = CPC * CS                 # 2048 q tokens / core
TCTX = CPC * TK               # 8192 ctx tokens / core
INNER = H * DH                # 512
QG = 4                        # chunks per q-projection group (N=256)
NQG = CPC // QG               # 8 q groups / core


def _build_bass(cpc=CPC, num_devices=N_CORES, do_rope=True, do_attn=True, do_out=True, attn_stop=3):
    tq = cpc * CS
    tctx = cpc * TK
    nqg = cpc // QG
    nc = bacc.Bacc("TRN2", target_bir_lowering=False, debug=False,
                   num_devices=num_devices)

    xT = nc.dram_tensor("xT", (DIM, tq), F32, kind="ExternalInput")
    ctxT = nc.dram_tensor("ctxT", (DIM, tctx), F32, kind="ExternalInput")
    Wq = nc.dram_tensor("Wq", (DIM, INNER), F32, kind="ExternalInput")   # pre-scaled
    Wk = nc.dram_tensor("Wk", (DIM, INNER), F32, kind="ExternalInput")
    Wv = nc.dram_tensor("Wv", (DIM, INNER), F32, kind="ExternalInput")
    Wo = nc.dram_tensor("Wo", (INNER, DIM), F32, kind="ExternalInput")
    bo = nc.dram_tensor("bo", (DIM,), F32, kind="ExternalInput")
    cos_kT = nc.dram_tensor("cos_kT", (64, 128), F32, kind="ExternalInput")
    sin_kT = nc.dram_tensor("sin_kT", (64, 128), F32, kind="ExternalInput")
    Pm = nc.dram_tensor("Pm", (64, 64), F32, kind="ExternalInput")
    nullkT = nc.dram_tensor("nullkT", (64, 8), F32, kind="ExternalInput")
    nullv_aug = nc.dram_tensor("nullv_aug", (1, 8 * 65), F32, kind="ExternalInput")
    cos_q0 = nc.dram_tensor("cos_q0", (64, 1), F32, kind="ExternalInput")
    sin_q0s = nc.dram_tensor("sin_q0s", (64, 1), F32, kind="ExternalInput")
    out = nc.dram_tensor("out", (tq, DIM), F32, kind="ExternalOutput")

    with tile.TileContext(nc) as tc:
        with tc.tile_pool(name="consts", bufs=1) as cp_, \
             tc.tile_pool(name="wk", bufs=2) as wk, \
             tc.tile_pool(name="psb", bufs=3, space="PSUM") as psb, \
             tc.tile_pool(name="pst", bufs=1, space="PSUM") as pst:

            # ---- constants ----
            wq_sb = cp_.tile([128, 8, INNER], F32R)
            nc.sync.dma_start(out=wq_sb, in_=Wq[:, :].rearrange(
                "(dt p) i -> p dt i", p=128).bitcast(F32R))
            wk_sb = cp_.tile([128, 8, INNER], F32R)
            nc.sync.dma_start(out=wk_sb, in_=Wk[:, :].rearrange(
                "(dt p) i -> p dt i", p=128).bitcast(F32R))
            wv_sb = cp_.tile([128, 8, INNER], F32R)
            nc.sync.dma_start(out=wv_sb, in_=Wv[:, :].rearrange(
                "(dt p) i -> p dt i", p=128).bitcast(F32R))
            wo_sb = cp_.tile([128, 4, DIM], F32R)
            nc.sync.dma_start(out=wo_sb, in_=Wo[:, :].rearrange(
                "(et p) c -> p et c", p=128).bitcast(F32R))

            bo_sb = cp_.tile([128, DIM], F32)
            nc.sync.dma_start(out=bo_sb, in_=bass.AP(
                tensor=bo, offset=0, ap=[[0, 128], [1, DIM]]))

            cosk_sb = cp_.tile([64, 128], F32)
            nc.sync.dma_start(out=cosk_sb, in_=cos_kT[:, :])
            sink_sb = cp_.tile([64, 128], F32)
            nc.sync.dma_start(out=sink_sb, in_=sin_kT[:, :])
            cosq_sb = cp_.tile([64, 1], F32)
            nc.sync.dma_start(out=cosq_sb, in_=cos_q0[:, :])
            sinq_sb = cp_.tile([64, 1], F32)
            nc.sync.dma_start(out=sinq_sb, in_=sin_q0s[:, :])

            pm_f32 = cp_.tile([64, 64], F32)
            nc.sync.dma_start(out=pm_f32, in_=Pm[:, :])
            pm_bf = cp_.tile([64, 64], BF16)
            nc.vector.tensor_copy(pm_bf, pm_f32)

            nullk_f32 = cp_.tile([64, 8], F32)
            nc.sync.dma_start(out=nullk_f32, in_=nullkT[:, :])
            nullk_bf = cp_.tile([64, 8], BF16)
            nc.vector.tensor_copy(nullk_bf, nullk_f32)

            nullv_f32 = cp_.tile([1, 8, 65], F32)
            nc.sync.dma_start(out=nullv_f32, in_=nullv_aug[:, :].rearrange(
                "o (h w) -> o h w", h=8))
            nullv_bf = cp_.tile([1, 8, 65], BF16)
            nc.vector.tensor_copy(nullv_bf, nullv_f32)

            ident = cp_.tile([128, 128], F32)
            make_identity(nc, ident)

            for g in range(nqg):          # 8 groups of 4 chunks
                # ---- q projection for this group: qT [512, 256] ----
                xT_sb = wk.tile([128, 8, QG * CS], F32R, tag="xT", bufs=1)
                nc.sync.dma_start(out=xT_sb, in_=xT[:, :].rearrange(
                    "(dt p) t -> p dt t", p=128)[:, :, g * QG * CS:(g + 1) * QG * CS]
                    .bitcast(F32R))
                qps = psb.tile([128, 4, QG * CS], F32, tag="ps", name=f"qps{g}")
                for it in range(4):
                    for dt in range(8):
                        nc.tensor.matmul(
                            qps[:, it, :],
                            wq_sb[:, dt, it * 128:(it + 1) * 128],
                            xT_sb[:, dt, :],
                            start=(dt == 0), stop=(dt == 7))
                qT_sb = wk.tile([64, 8, QG * CS], BF16, tag="qT", bufs=2)
                for it in range(4):
                    nc.vector.tensor_copy(qT_sb[:, 2 * it, :], qps[0:64, it, :])
                    nc.vector.tensor_copy(qT_sb[:, 2 * it + 1, :], qps[64:128, it, :])
                # rope-q: fix token 0 of each chunk (cols ::CS)
                qcols = qT_sb[:, :, :].rearrange(
                    "p h (c w) -> p h c w", w=CS)[:, :, :, 0]   # [64, 8, QG]
                t1q = wk.tile([64, 8, QG], BF16, tag="t1q", bufs=2)
                nc.vector.tensor_mul(
                    t1q, qcols,
                    cosq_sb.unsqueeze(2).broadcast_to((64, 8, QG)))
                t2q = wk.tile([64, 8, QG], BF16, tag="t2q", bufs=2)
                for (dst, src) in ((0, 32), (32, 0)):
                    nc.vector.tensor_mul(
                        t2q[dst:dst + 32, :, :],
                        qT_sb[:, :, :].rearrange(
                            "p h (c w) -> p h c w", w=CS)[src:src + 32, :, :, 0],
                        sinq_sb[src:src + 32, :].unsqueeze(2)
                        .broadcast_to((32, 8, QG)))
                nc.vector.tensor_add(qcols, t1q, t2q)

                # ---- null sims for group: expn_g [1, 8, 256] bf16 ----
                expn_g = wk.tile([1, 8, QG * CS], BF16, tag="expn", bufs=2)
                for h in range(H):
                    nps = pst.tile([1, QG * CS], F32, tag="pst", name=f"nps{g}_{h}")
                    nc.tensor.matmul(
                        nps[:, :],
                        nullk_bf[:, h:h + 1],
                        qT_sb[:, h, :],
                        start=True, stop=True)
                    nc.scalar.activation(expn_g[:, h, :], nps[:, :],
                                         mybir.ActivationFunctionType.Exp)

                for cc in range(QG):
                    c = g * QG + cc       # chunk index within core
                    # ---- load ctxT slice [1024, 256] ----
                    ctx_sb = wk.tile([128, 8, TK], F32R, tag="ctx", bufs=2)
                    nc.sync.dma_start(out=ctx_sb, in_=ctxT[:, :].rearrange(
                        "(dt p) t -> p dt t", p=128)[:, :, c * TK:(c + 1) * TK]
                        .bitcast(F32R))

                    # ---- k projection -> kraw bf16 ----
                    kps = psb.tile([128, 4, TK], F32, tag="ps", name=f"kps{c}")
                    for it in range(4):
                        for dt in range(8):
                            nc.tensor.matmul(
                                kps[:, it, :],
                                wk_sb[:, dt, it * 128:(it + 1) * 128],
                                ctx_sb[:, dt, :],
                                start=(dt == 0), stop=(dt == 7))
                    kraw = wk.tile([64, 8, TK], BF16, tag="kraw", bufs=2)
                    for it in range(4):
                        nc.scalar.copy(kraw[:, 2 * it, :], kps[0:64, it, :])
                        nc.scalar.copy(kraw[:, 2 * it + 1, :], kps[64:128, it, :])

                    # ---- rope-k: perm matmul + combine ----
                    if not do_rope:
                        kT_bf = kraw
                    else:
                      kpps_a = psb.tile([64, 4, TK], F32, tag="ps", name=f"kppsa{c}")
                      kpps_b = psb.tile([64, 4, TK], F32, tag="ps", name=f"kppsb{c}")
                      for q4 in range(4):
                        dst_t = (kpps_a, kpps_b)[q4 // 2]
                        nc.tensor.matmul(
                            dst_t[:, :, :].rearrange("p h t -> p (h t)")
                            [:, (q4 % 2) * 512:(q4 % 2 + 1) * 512],
                            pm_bf,
                            kraw[:, :, :].rearrange("p h t -> p (h t)")
                            [:, q4 * 512:(q4 + 1) * 512],
                            start=True, stop=True)
                      t1k = wk.tile([64, 8, TK], BF16, tag="t1k", bufs=1)
                      nc.vector.tensor_mul(
                        t1k[:, :, :].rearrange("p h (rep c) -> p h rep c", rep=2),
                        kraw[:, :, :].rearrange("p h (rep c) -> p h rep c", rep=2),
                        cosk_sb.unsqueeze(1).unsqueeze(2)
                        .broadcast_to((64, 8, 2, 128)))
                      t2k = wk.tile([64, 8, TK], BF16, tag="t2k", bufs=1)
                      for half, kp_t in ((0, kpps_a), (1, kpps_b)):
                        nc.vector.tensor_mul(
                            t2k[:, half * 4:(half + 1) * 4, :].rearrange(
                                "p h (rep c) -> p h rep c", rep=2),
                            kp_t[:, :, :].rearrange(
                                "p h (rep c) -> p h rep c", rep=2),
                            sink_sb.unsqueeze(1).unsqueeze(2)
                            .broadcast_to((64, 4, 2, 128)))
                      kT_bf = wk.tile([64, 8, TK], BF16, tag="kT", bufs=2)
                      nc.vector.tensor_add(kT_bf, t1k, t2k)

                    # ---- v projection -> v_aug bf16 [128, 2, 8, 65] ----
                    vps = psb.tile([128, 2, INNER], F32, tag="ps", name=f"vps{c}")
                    for tg in range(2):
                        for dt in range(8):
                            nc.tensor.matmul(
                                vps[:, tg, :],
                                ctx_sb[:, dt, tg * 128:(tg + 1) * 128],
                                wv_sb[:, dt, :],
                                start=(dt == 0), stop=(dt == 7))
                    v_aug = wk.tile([128, 2, 8, 65], BF16, tag="v_aug", bufs=2)
                    nc.scalar.copy(
                        v_aug[:, :, :, 0:64],
                        vps[:, :, :].rearrange("p tg (h w) -> p tg h w", h=8))
                    nc.gpsimd.memset(v_aug[:, :, :, 64:65], 1.0)

                    if not do_attn:
                        continue
                    # ---- sim matmuls: simT [128j, 2jg, (h,i)] ----
                    sps = psb.tile([128, 2, 512], F32, tag="ps", name=f"sps{c}")
                    for h in range(H):
                        for jg in range(2):
                            nc.tensor.matmul(
                                sps[:, jg, h * 64:(h + 1) * 64],
                                kT_bf[:, h, jg * 128:(jg + 1) * 128],
                                qT_sb[:, h, cc * CS:(cc + 1) * CS],
                                start=True, stop=True)
                    if attn_stop == 0:
                        dbg = wk.tile([64, DIM], F32, tag="out_sb", bufs=2)
                        nc.vector.tensor_copy(dbg[:, 0:512], sps[0:64, 0, :])
                        nc.vector.memset(dbg[:, 512:], 0.0)
                        nc.sync.dma_start(out=out[c * CS:(c + 1) * CS, :], in_=dbg)
                        continue
                    expT = wk.tile([128, 2, 512], BF16, tag="expT", bufs=2)
                    nc.scalar.activation(expT, sps,
                                         mybir.ActivationFunctionType.Exp)
                    if attn_stop == 1:
                        dbg = wk.tile([64, DIM], F32, tag="out_sb", bufs=2)
                        nc.vector.tensor_copy(dbg[:, 0:512], expT[0:64, 0, :])
                        nc.vector.memset(dbg[:, 512:], 0.0)
                        nc.sync.dma_start(out=out[c * CS:(c + 1) * CS, :], in_=dbg)
                        continue

                    # ---- o matmuls [64i, 65] per head (col 64 = softmax sum) ----
                    ops_ = psb.tile([64, 8, 128], F32, tag="ps", name=f"ops{c}")
                    for h in range(H):
                        dst = ops_[:, h, 0:65]
                        for jg in range(2):
                            nc.tensor.matmul(
                                dst,
                                expT[:, jg, h * 64:(h + 1) * 64],
                                v_aug[:, jg, h, :],
                                start=(jg == 0), stop=False)
                        nc.tensor.matmul(
                            dst,
                            expn_g[0:1, h, c * CS - g * QG * CS:
                                   c * CS - g * QG * CS + CS],
                            nullv_bf[0:1, h, :],
                            start=False, stop=True)

                    if attn_stop == 2:
                        dbg = wk.tile([64, DIM], F32, tag="out_sb", bufs=2)
                        nc.vector.tensor_copy(dbg[:, 0:128], ops_[:, 0, :])
                        nc.vector.memset(dbg[:, 128:], 0.0)
                        nc.sync.dma_start(out=out[c * CS:(c + 1) * CS, :], in_=dbg)
                        continue
                    # ---- normalize: o_sb [64, 8, 64] fp32 ----
                    rcol = wk.tile([64, 8], F32, tag="rcol", bufs=2)
                    nc.vector.reciprocal(rcol, ops_[:, :, 64])
                    o_sb = wk.tile([64, 8, 64], F32, tag="o_sb", bufs=2)
                    for h in range(H):
                        nc.vector.tensor_scalar_mul(
                            o_sb[:, h, :],
                            ops_[:, h, 0:64],
                            rcol[:, h:h + 1])

                    if not do_out:
                        continue
                    # ---- transpose o -> oT fp32r [128e, 4et, 64t] ----
                    otr = pst.tile([128, 4, 64], F32, tag="pst", name=f"otr{c}")
                    for et in range(4):
                        nc.tensor.transpose(
                            otr[:, et, :],
                            o_sb[:, 2 * et:2 * et + 2, :],
                            ident[0:64, 0:64])
                    oT_sb = wk.tile([128, 4, 64], F32R, tag="oT", bufs=2)
                    nc.vector.tensor_copy(oT_sb, otr)

                    # ---- out projection + bias ----
                    outps = psb.tile([64, DIM], F32, tag="ps", name=f"outps{c}")
                    for co in range(2):
                        for et in range(4):
                            nc.tensor.matmul(
                                outps[:, co * 512:(co + 1) * 512],
                                oT_sb[:, et, :],
                                wo_sb[:, et, co * 512:(co + 1) * 512],
                                start=(et == 0), stop=(et == 3))
                    out_sb = wk.tile([64, DIM], F32, tag="out_sb", bufs=2)
                    nc.vector.tensor_add(out_sb, outps, bo_sb)
                    nc.sync.dma_start(out=out[c * CS:(c + 1) * CS, :], in_=out_sb)

    nc.compile()
    return nc


_CACHED_NC = None


def _get_nc():
    global _CACHED_NC
    if _CACHED_NC is None:
        _CACHED_NC = _build_bass()
    return _CACHED_NC


def kernel(x, context, q_pos_emb, k_pos_emb, Wq, Wk, Wv, Wo, bo, null_k, null_v):
    x = np.asarray(x, dtype=np.float32)
    context = np.asarray(context, dtype=np.float32)
    q_pos_emb = np.asarray(q_pos_emb, dtype=np.float32)
    k_pos_emb = np.asarray(k_pos_emb, dtype=np.float32)
    Wq = np.asarray(Wq, dtype=np.float32)
    Wk = np.asarray(Wk, dtype=np.float32)
    Wv = np.asarray(Wv, dtype=np.float32)
    Wo = np.asarray(Wo, dtype=np.float32)
    bo = np.asarray(bo, dtype=np.float32)
    null_k = np.asarray(null_k, dtype=np.float32)
    null_v = np.asarray(null_v, dtype=np.float32)

    # ---- host marshalling (layout only + tiny rope tables) ----
    xs = np.zeros_like(x)
    xs[:, : N - CP] = x[:, CP:]
    xc = xs.reshape(BK, CS, DIM)
    ctx = context.reshape(BK, TK, DIM)

    Wq_s = np.ascontiguousarray(Wq * SCALE)

    qpe63 = q_pos_emb[0, 0, CP]
    cos_q0 = np.cos(qpe63)[:, None].astype(np.float32)          # [64, 1]
    sgn = np.where(np.arange(64) < 32, -1.0, 1.0)
    sin_q0s = (np.sin(qpe63) * sgn)[:, None].astype(np.float32)
    # permuted so the partition-shifted mul reads table at the src base
    # partition (BIR requires equal base partitions for two SBUF inputs)
    sp = np.empty_like(sin_q0s)
    sp[0:32] = sin_q0s[32:64]; sp[32:64] = sin_q0s[0:32]
    sin_q0s = sp

    kpe = k_pos_emb[0, 0]
    cos_kT = np.ascontiguousarray(np.cos(kpe.T).astype(np.float32))   # [64, 128]
    sin_kT = np.ascontiguousarray(np.sin(kpe.T).astype(np.float32))

    Pm = np.zeros((64, 64), np.float32)
    for rout in range(64):
        if rout < 32:
            Pm[rout + 32, rout] = -1.0
        else:
            Pm[rout - 32, rout] = 1.0

    nullkT = np.ascontiguousarray(null_k.reshape(8, 64).T.astype(np.float32))  # [64, 8]
    nullv_aug = np.zeros((1, 8, 65), np.float32)
    nullv_aug[0, :, :64] = null_v.reshape(8, 64)
    nullv_aug[0, :, 64] = 1.0
    nullv_aug = nullv_aug.reshape(1, 8 * 65)

    shared = {
        "Wq": Wq_s, "Wk": Wk, "Wv": Wv, "Wo": Wo, "bo": bo,
        "cos_kT": cos_kT, "sin_kT": sin_kT, "Pm": Pm,
        "nullkT": nullkT, "nullv_aug": nullv_aug,
        "cos_q0": cos_q0, "sin_q0s": sin_q0s,
    }
    in_maps = []
    for c in range(N_CORES):
        sl = slice(c * CPC, (c + 1) * CPC)
        xT_c = np.ascontiguousarray(xc[sl].reshape(TQ, DIM).T)
        ctxT_c = np.ascontiguousarray(ctx[sl].reshape(TCTX, DIM).T)
        in_maps.append({"xT": xT_c, "ctxT": ctxT_c, **shared})

    nc = _get_nc()
    res = run_bass_kernel_spmd(nc, in_maps, core_ids=list(range(N_CORES)))

    out_full = np.concatenate([res.results[c]["out"] for c in range(N_CORES)],
                              axis=0)                      # [BK*CS, DIM]
    o = out_full.reshape(B, K_CHUNKS * CS, DIM)
    final = np.concatenate(
        [np.zeros((B, CP, DIM), np.float32), o[:, : K_CHUNKS * CS - CP]], axis=1)
    return final
